# revision 13
# baseline (speedup 1.0000x reference)
"""Trainium2 Bass kernel for nn_HardQuadTripletSOSRLoss.

Sharding: 8 cores = 2 batches x 4 HW-shards (4096 grid cells each).

Device (per core): 32 bf16 matmuls kp1_desc[b].T x desc2-shard -> PSUM f32
scores [512 rows x 4096 cells], drained by two engines in parallel on
disjoint cell ranges:
  - DVE: max8 top-8 values per [128,1024]-cell unit (exact top-8/unit),
  - Act: PSUM -> fp8(e4m3) sketch, DMA'd out for host-side selection.
Host: bilinear sampling, geometry/masks, SOS top-8 (small: 6% of flops),
fp8-candidate exact re-evaluation, certificate-checked merge with exact
row-repair fallback.
"""

import os

import numpy as np
import ml_dtypes

import concourse.mybir as mybir
import concourse.tile as tile
from concourse import bacc
from concourse.bass_utils import run_bass_kernel_spmd

# ---- problem constants (hardcoded per contract) ----
B, N, C, H, W = 2, 512, 128, 128, 128
HW = H * W
GS = 8
NUM_NEG = 16
SOS_NEG = 8
MARGIN = 1.0
NSHARD = 4
SHW = HW // NSHARD          # 4096 cells per shard
RT = N // 128               # 4 row tiles
UW = 1024                   # unit width (cells) = 2 PSUM banks
NU = SHW // UW              # 4 units per shard (h = 0..3)

# Route map: per row-tile t, first D_CELLS[t] cells of the shard go to the
# DVE (exact top-8 per 1024-unit); the rest are fp8-sketched for the host.
D_UNITS = [2, 2, 2, 2]      # units routed to DVE per row tile (h < D_UNITS[t])
D_CELLS = [u * UW for u in D_UNITS]
F8W = SHW - min(D_CELLS)    # fp8 tensor width (max over t): 3072

F32 = mybir.dt.float32
BF16 = mybir.dt.bfloat16
F8 = mybir.dt.float8e4

_NC_CACHE = {}
LAST_RESULTS = None  # BassKernelResults of most recent device run (for test.py)


def _build_nc():
    nc = bacc.Bacc("TRN2", target_bir_lowering=False, debug=False, num_devices=8)

    lhsT = nc.dram_tensor("lhsT", [C, N], BF16, kind="ExternalInput")
    rhs = nc.dram_tensor("rhs", [C, SHW], BF16, kind="ExternalInput")
    cand = nc.dram_tensor("cand", [128, RT * 16], F32, kind="ExternalOutput")
    f8 = nc.dram_tensor("f8", [RT, 128, F8W], F8, kind="ExternalOutput")

    with tile.TileContext(nc) as tc:
        with (
            tc.tile_pool(name="const", bufs=1) as cpool,
            tc.tile_pool(name="cnd", bufs=1) as cndpool,
            tc.tile_pool(name="sk", bufs=2) as skpool,
            tc.tile_pool(name="psum", bufs=4, space="PSUM") as pspool,
        ):
            # Input DMAs spread across sequencers so descriptor gen runs in
            # parallel right after the framework preamble barrier.
            lhsT_sb = cpool.tile([C, N], BF16, tag="lhsT")
            nc.sync.dma_start(lhsT_sb[:], lhsT[:, :])
            rhs_sb = cpool.tile([C, SHW], BF16, tag="rhs")
            dma_eng = {0: nc.scalar, 2: nc.sync, 1: nc.scalar, 3: nc.gpsimd}
            for h in (0, 2, 1, 3):
                dma_eng[h].dma_start(
                    rhs_sb[:, h * UW : (h + 1) * UW], rhs[:, h * UW : (h + 1) * UW]
                )

            cn = cndpool.tile([128, RT * 16], F32, tag="cn")

            # Interleave DVE-routed and Act-routed units so the engines
            # drain PSUM in parallel; t-major so fp8 DMAs fire early. The
            # final units are DVE-routed so the kernel ends on the cheap
            # cand DMA. Unit = (t, h) covers cells [h*UW, (h+1)*UW) with
            # lhsT rows [t*128, (t+1)*128).
            dve_units = [(t, h) for t in range(RT) for h in (0, 1) if h < D_UNITS[t]]
            act_units = [(t, h) for t in range(RT) for h in (2, 3) if h >= D_UNITS[t]]
            order = []
            for du, au in zip(dve_units, act_units):
                order.append(("A", au))
                order.append(("D", du))

            for kind, (t, h) in order:
                ps = pspool.tile([128, UW], F32, tag="u")
                for j in range(2):
                    c0 = h * UW + j * 512
                    nc.tensor.matmul(
                        ps[:, j * 512 : (j + 1) * 512],
                        lhsT_sb[:, t * 128 : (t + 1) * 128],
                        rhs_sb[:, c0 : c0 + 512],
                        start=True,
                        stop=True,
                    )
                if kind == "D":
                    nc.vector.max(cn[:, t * 16 + h * 8 : t * 16 + (h + 1) * 8], ps[:])
                else:
                    sk = skpool.tile([128, UW], F8, tag="sk")
                    nc.scalar.copy(sk[:], ps[:])
                    off = (h - D_UNITS[t]) * UW
                    # per-unit fp8 DMA on the idle Pool DGE
                    nc.gpsimd.dma_start(f8[t, :, off : off + UW], sk[:])

            nc.sync.dma_start(cand[:, :], cn[:])

    nc.compile()
    return nc


def _get_nc():
    if "nc" not in _NC_CACHE:
        _NC_CACHE["nc"] = _build_nc()
    return _NC_CACHE["nc"]


# ---------------- host-side helpers (all float32, mirror reference) ----------


def _sample_descriptors(desc2, kp):
    """Bilinear sample of desc2 (B,C,H,W) at image-space (y,x) kp, L2-normed."""
    b, c, h, w = desc2.shape
    f = np.float32
    y = np.clip(kp[..., 0] / f(GS) - f(0.5), f(0.0), f(h - 1.0)).astype(f)
    x = np.clip(kp[..., 1] / f(GS) - f(0.5), f(0.0), f(w - 1.0)).astype(f)
    y0 = np.clip(np.floor(y), 0, h - 2).astype(np.int64)
    x0 = np.clip(np.floor(x), 0, w - 2).astype(np.int64)
    wy = (y - y0.astype(f))[..., None]
    wx = (x - x0.astype(f))[..., None]
    dmap = desc2.transpose(0, 2, 3, 1).reshape(b, h * w, c)

    def g(yi, xi):
        idx = yi * w + xi
        return np.take_along_axis(dmap, idx[..., None], axis=1)

    v = (
        g(y0, x0) * (1 - wy) * (1 - wx)
        + g(y0, x0 + 1) * (1 - wy) * wx
        + g(y0 + 1, x0) * wy * (1 - wx)
        + g(y0 + 1, x0 + 1) * wy * wx
    )
    n = np.sqrt(np.sum(v * v, axis=-1, keepdims=True)).astype(f)
    return (v / (n + f(1e-8))).astype(f)


def _nearest4(pts):
    """Flat ids (..., 4) of the 4 nearest grid-cell centers, matching the
    reference's top_k over all HW cells (ties -> lower flat id)."""
    f = np.float32
    y = pts[..., 0]
    x = pts[..., 1]
    cy = np.clip(np.floor(y / f(GS)).astype(np.int64), 0, H - 1)
    cx = np.clip(np.floor(x / f(GS)).astype(np.int64), 0, W - 1)
    by = np.clip(cy - 2, 0, H - 5)
    bx = np.clip(cx - 2, 0, W - 5)
    offs = np.arange(5, dtype=np.int64)
    iy = by[..., None] + offs          # (..., 5)
    ix = bx[..., None] + offs
    cyc = (f(GS) * iy + f(GS / 2.0)).astype(f)
    cxc = (f(GS) * ix + f(GS / 2.0)).astype(f)
    dy = y[..., None] - cyc
    dx = x[..., None] - cxc
    d2 = (dy * dy)[..., :, None] + (dx * dx)[..., None, :]   # (..., 5, 5)
    ids = iy[..., :, None] * W + ix[..., None, :]
    d2 = d2.reshape(d2.shape[:-2] + (25,))
    ids = ids.reshape(ids.shape[:-2] + (25,))
    order = np.argsort(d2, axis=-1, kind="stable")[..., :4]
    return np.take_along_axis(ids, order, axis=-1)


def _warp(p, Hm):
    f = np.float32
    xy = p[..., ::-1]
    ph = np.concatenate([xy, np.ones_like(xy[..., :1])], axis=-1)
    wp = np.einsum("bij,bmj->bmi", Hm, ph).astype(f)
    wp = wp[..., :2] / (wp[..., 2:3] + f(1e-8))
    return wp[..., ::-1].astype(f)


def _centers(ids):
    f = np.float32
    yy = (ids // W).astype(f) * f(GS) + f(GS / 2.0)
    xx = (ids % W).astype(f) * f(GS) + f(GS / 2.0)
    return np.stack([yy, xx], axis=-1)


def _topk_smallest_idx(x, k):
    """Indices of k smallest per row, ties -> lower index (lax.top_k order)."""
    return np.argsort(x, axis=-1, kind="stable")[..., :k]


def kernel(kp1, w_kp1, kp1_desc, desc2, homo12):
    global LAST_RESULTS

    f = np.float32
    kp1 = np.asarray(kp1, f)
    w_kp1 = np.asarray(w_kp1, f)
    kp1_desc = np.asarray(kp1_desc, f)
    desc2 = np.asarray(desc2, f)
    homo12 = np.asarray(homo12, f)

    # ---------------- host geometry / small tensors ----------------
    w_kp1_desc = _sample_descriptors(desc2, w_kp1)                  # (B,N,C)
    pos = f(2.0) - f(2.0) * np.einsum("bnc,bnc->bn", kp1_desc, w_kp1_desc)

    cell4 = _nearest4(kp1)                                          # (B,N,4)
    kp1_cells = _centers(cell4.reshape(B, 4 * N))                   # (B,4N,2)
    warped = _warp(kp1_cells, homo12)                               # (B,4N,2)
    wcc = _nearest4(warped)                                         # (B,4N,4)
    ids16 = wcc.reshape(B, N, 16)                                   # neigh cells
    cell4_w = _nearest4(w_kp1)                                      # (B,N,4)

    eqk = cell4[:, :, :, None, None] == cell4[:, None, None, :, :]
    kp1_mask = eqk.sum(axis=(2, 4)).astype(f)                       # (B,N,N)
    eqw = ids16[:, :, :, None, None] == cell4_w[:, None, None, :, :]
    w_kp1_mask = eqw.sum(axis=(2, 4)).astype(f)                     # (B,N,N)

    # ---------------- device run ----------------
    nc = _get_nc()
    bf = ml_dtypes.bfloat16
    desc2_flat = np.ascontiguousarray(desc2.reshape(B, C, HW))
    in_maps = []
    for b in range(B):
        lhsT_b = np.ascontiguousarray(kp1_desc[b].T.astype(bf))
        for s in range(NSHARD):
            in_maps.append(
                {
                    "lhsT": lhsT_b,
                    "rhs": np.ascontiguousarray(
                        desc2_flat[b][:, s * SHW : (s + 1) * SHW].astype(bf)
                    ),
                }
            )
    want_trace = bool(int(os.environ.get("KT_TRACE", "0")))
    try:
        res = run_bass_kernel_spmd(
            nc, in_maps, core_ids=list(range(8)), trace=want_trace
        )
    except ModuleNotFoundError:
        res = run_bass_kernel_spmd(nc, in_maps, core_ids=list(range(8)), trace=False)
    LAST_RESULTS = res
    results = res.results

    # dve_vals[b, n, s, :] : top-8 values per DVE unit (t0-2: 2 units, t3: 1)
    # f8vals[b, t, s]      : fp8 sketch [128, W_t] (as float32)
    dve_vals = np.full((B, N, NSHARD, 16), -np.inf, f)
    f8vals = np.empty((B, RT, NSHARD, 128, F8W), f)
    for ci in range(8):
        b, s = divmod(ci, NSHARD)
        r = results[ci]
        cnd = np.asarray(r["cand"], f)                              # (128,RT*16)
        skv = np.asarray(r["f8"]).astype(f)                         # (RT,128,F8W)
        for t in range(RT):
            rows = slice(t * 128, (t + 1) * 128)
            nu_d = D_UNITS[t]
            dve_vals[b, rows, s, : nu_d * 8] = cnd[:, t * 16 : t * 16 + nu_d * 8]
            f8vals[b, t, s] = skv[t]

    # ---------------- fos merge ----------------
    # exact raw sims of the 16 neighbor (masked) cells
    hwdesc = desc2_flat.transpose(0, 2, 1)                          # (B,HW,C)
    gath = np.take_along_axis(
        hwdesc, ids16.reshape(B, N * 16)[:, :, None], axis=1
    ).reshape(B, N, 16, C)
    vm16 = np.einsum("bnc,bnjc->bnj", kp1_desc, gath).astype(f)     # (B,N,16)

    KF = 96                    # fp8 candidates to exact-evaluate per row
    NOISE = f(0.008)           # bf16-input matmul noise bound on scores
    neg_scores = np.empty((B, N, NUM_NEG), f)
    repair = np.zeros((B, N), bool)

    # Per-row fp8 candidate cells (global ids) + non-candidate upper bound.
    # Row n (in tile t): fp8 region of shard s = cells
    # [s*SHW + D_CELLS[t], (s+1)*SHW) with width Wt = SHW - D_CELLS[t].
    for t in range(RT):
        rows = slice(t * 128, (t + 1) * 128)
        Wt = SHW - D_CELLS[t]
        blk = f8vals[:, t, :, :, :Wt]                               # (B,S,128,Wt)
        blk = blk.transpose(0, 2, 1, 3).reshape(B, 128, NSHARD * Wt)
        # global cell ids for the concatenated fp8 region
        ids_s = (
            np.arange(NSHARD)[:, None] * SHW + D_CELLS[t] + np.arange(Wt)[None, :]
        ).reshape(-1)                                               # (S*Wt,)
        top = np.argpartition(-blk, KF, axis=-1)[..., : KF + 1]
        topv = np.take_along_axis(blk, top, axis=-1)
        ordi = np.argsort(-topv, axis=-1, kind="stable")
        top = np.take_along_axis(top, ordi, axis=-1)
        topv = np.take_along_axis(topv, ordi, axis=-1)
        cand_ids = ids_s[top[..., :KF]]                             # (B,128,KF)
        b_max = topv[..., KF]                                       # max non-cand fp8
        ub = b_max + np.maximum(np.abs(b_max), f(0.25)) * f(0.13) + NOISE

        # exact values of fp8 candidates
        gat = np.take_along_axis(
            hwdesc, cand_ids.reshape(B, 128 * KF)[:, :, None], axis=1
        ).reshape(B, 128, KF, C)
        ev = np.einsum("bnc,bnkc->bnk", kp1_desc[:, rows], gat).astype(f)

        # masked-cell adjustment inside the fp8 region (by index, exact)
        ids16_t = ids16[:, rows]                                    # (B,128,16)
        for bb in range(B):
            for rr in range(128):
                n = t * 128 + rr
                e = ev[bb, rr].copy()
                cid = cand_ids[bb, rr]
                uq, cnts = np.unique(ids16_t[bb, rr], return_counts=True)
                in_dve = (uq % SHW) < D_CELLS[t]
                # fp8-region masked cells present among candidates
                for u, cc in zip(uq[~in_dve], cnts[~in_dve]):
                    hit = np.nonzero(cid == u)[0]
                    if hit.size:
                        e[hit[0]] -= f(2.5) * cc
                # DVE-region masked cells: patch by value or repair
                dv = dve_vals[bb, n].reshape(-1).copy()
                order_mask = np.argsort(-vm16[bb, n])
                for j in order_mask:
                    u = ids16_t[bb, rr, j]
                    if (u % SHW) >= D_CELLS[t]:
                        continue
                    # count of this cell (dedupe: only process first occurrence)
                    if j != np.nonzero(ids16_t[bb, rr] == u)[0][0]:
                        continue
                    cc = int((ids16_t[bb, rr] == u).sum())
                    s_of = u // SHW
                    unit = (u % SHW) // UW
                    u8 = dve_vals[bb, n, s_of, unit * 8 : unit * 8 + 8]
                    vm = vm16[bb, n, j]
                    if vm + NOISE + f(0.004) < u8[7]:
                        continue                    # below the unit's top-8
                    d = np.abs(u8 - vm)
                    hits = np.nonzero(d <= NOISE)[0]
                    if hits.size != 1 or np.sort(d)[1] <= f(2.0) * NOISE:
                        repair[bb, n] = True
                        break
                    dv[s_of * 16 + unit * 8 + hits[0]] -= f(2.5) * cc
                if repair[bb, n]:
                    continue
                pool_v = np.concatenate([e, dv[np.isfinite(dv)]])
                pool_v.sort()
                top16 = pool_v[::-1][:NUM_NEG]
                v16 = top16[-1]
                # certificates
                if ub[bb, rr] >= v16 - f(1e-3):
                    repair[bb, n] = True
                    continue
                unit8 = dve_vals[bb, n, :, 7::8].reshape(-1)        # 8th per unit
                unit8 = unit8[np.isfinite(unit8)]
                if unit8.size and unit8.max() + NOISE >= v16:
                    repair[bb, n] = True
                    continue
                neg_scores[bb, n] = top16

    # exact repair (rare): full masked row on host
    if os.environ.get("KT_DEBUG"):
        print(f"[kernel] repair rows: {int(repair.sum())} / {B * N}")
    rep_idx = np.argwhere(repair)
    if rep_idx.size:
        for bb, n in rep_idx:
            row = hwdesc[bb] @ kp1_desc[bb, n]                      # (HW,)
            np.subtract.at(row, ids16[bb, n], f(2.5))
            neg_scores[bb, n] = np.sort(row)[::-1][:NUM_NEG]

    neg = f(2.0) - f(2.0) * neg_scores                              # (B,N,16)
    fos = np.mean(
        np.maximum(pos[..., None] - neg + f(MARGIN), f(0.0)) ** 2
    ).astype(f)

    # ---------------- sos (host: 6% of total flops) ----------------
    k_sim = (
        f(2.0) - f(2.0) * np.einsum("bnc,bmc->bnm", kp1_desc, kp1_desc)
        + f(5.0) * kp1_mask
    ).astype(f)
    w_sim = (
        f(2.0) - f(2.0) * np.einsum("bnc,bmc->bnm", w_kp1_desc, w_kp1_desc)
        + f(5.0) * w_kp1_mask
    ).astype(f)
    k_ids = _topk_smallest_idx(k_sim, SOS_NEG)                      # (B,N,8)
    w_ids = _topk_smallest_idx(w_sim, SOS_NEG)

    kd = np.take_along_axis(
        kp1_desc, k_ids.reshape(B, N * SOS_NEG)[:, :, None], axis=1
    ).reshape(B, N, SOS_NEG, C)
    wd = np.take_along_axis(
        w_kp1_desc, w_ids.reshape(B, N * SOS_NEG)[:, :, None], axis=1
    ).reshape(B, N, SOS_NEG, C)
    a = f(2.0) - f(2.0) * np.einsum("bnc,bnkc->bnk", kp1_desc, kd)
    bb_ = f(2.0) - f(2.0) * np.einsum("bnc,bnkc->bnk", w_kp1_desc, wd)
    sv = (a - bb_).astype(f)
    sos = np.mean(np.sqrt(np.sum(sv * sv, axis=-1))).astype(f)

    return np.asarray(fos + sos, dtype=np.float32)


# revision 14
# speedup vs baseline: 1.0405x; 1.0405x over previous
"""Trainium2 Bass kernel for nn_HardQuadTripletSOSRLoss.

Sharding: 8 cores = 2 batches x 4 HW-shards (4096 grid cells each).

Device (per core): 32 bf16 matmuls kp1_desc[b].T x desc2-shard -> PSUM f32
scores [512 rows x 4096 cells], drained by two engines in parallel on
disjoint cell ranges:
  - DVE: max8 top-8 values per [128,1024]-cell unit (exact top-8/unit),
  - Act: PSUM -> fp8(e4m3) sketch, DMA'd out for host-side selection.
Host: bilinear sampling, geometry/masks, SOS top-8 (small: 6% of flops),
fp8-candidate exact re-evaluation, certificate-checked merge with exact
row-repair fallback.
"""

import os

import numpy as np
import ml_dtypes

import concourse.mybir as mybir
import concourse.tile as tile
from concourse import bacc
from concourse.bass_utils import run_bass_kernel_spmd

# ---- problem constants (hardcoded per contract) ----
B, N, C, H, W = 2, 512, 128, 128, 128
HW = H * W
GS = 8
NUM_NEG = 16
SOS_NEG = 8
MARGIN = 1.0
NSHARD = 4
SHW = HW // NSHARD          # 4096 cells per shard
RT = N // 128               # 4 row tiles
UW = 1024                   # unit width (cells) = 2 PSUM banks
NU = SHW // UW              # 4 units per shard (h = 0..3)

# Route map: per row-tile t, first D_CELLS[t] cells of the shard go to the
# DVE (exact top-8 per 1024-unit); the rest are fp8-sketched for the host.
D_UNITS = [2, 2, 2, 2]      # units routed to DVE per row tile (h < D_UNITS[t])
D_CELLS = [u * UW for u in D_UNITS]
F8W = SHW - min(D_CELLS)    # fp8 tensor width (max over t): 3072

F32 = mybir.dt.float32
BF16 = mybir.dt.bfloat16
F8 = mybir.dt.float8e4

_NC_CACHE = {}
LAST_RESULTS = None  # BassKernelResults of most recent device run (for test.py)


def _build_nc():
    nc = bacc.Bacc("TRN2", target_bir_lowering=False, debug=False, num_devices=8)

    lhsT = nc.dram_tensor("lhsT", [C, N], BF16, kind="ExternalInput")
    rhs = nc.dram_tensor("rhs", [C, SHW], BF16, kind="ExternalInput")
    cand = nc.dram_tensor("cand", [128, RT * 16], F32, kind="ExternalOutput")
    f8 = nc.dram_tensor("f8", [RT, 128, F8W], F8, kind="ExternalOutput")

    with tile.TileContext(nc) as tc:
        with (
            tc.tile_pool(name="const", bufs=1) as cpool,
            tc.tile_pool(name="cnd", bufs=1) as cndpool,
            tc.tile_pool(name="sk", bufs=2) as skpool,
            tc.tile_pool(name="psum", bufs=4, space="PSUM") as pspool,
        ):
            # Input DMAs spread across sequencers so descriptor gen runs in
            # parallel right after the framework preamble barrier.
            lhsT_sb = cpool.tile([C, N], BF16, tag="lhsT")
            nc.sync.dma_start(lhsT_sb[:], lhsT[:, :])
            rhs_sb = cpool.tile([C, SHW], BF16, tag="rhs")
            dma_eng = {0: nc.scalar, 2: nc.sync, 1: nc.scalar, 3: nc.gpsimd}
            for h in (0, 2, 1, 3):
                dma_eng[h].dma_start(
                    rhs_sb[:, h * UW : (h + 1) * UW], rhs[:, h * UW : (h + 1) * UW]
                )

            cn = cndpool.tile([128, RT * 16], F32, tag="cn")

            # Interleave DVE-routed and Act-routed units so the engines
            # drain PSUM in parallel; t-major so fp8 DMAs fire early. The
            # final units are DVE-routed so the kernel ends on the cheap
            # cand DMA. Unit = (t, h) covers cells [h*UW, (h+1)*UW) with
            # lhsT rows [t*128, (t+1)*128).
            dve_units = [(t, h) for t in range(RT) for h in (0, 1) if h < D_UNITS[t]]
            act_units = [(t, h) for t in range(RT) for h in (2, 3) if h >= D_UNITS[t]]
            order = []
            for i, (du, au) in enumerate(zip(dve_units, act_units)):
                if i < len(dve_units) - 1:
                    order.append(("D", du))
                    order.append(("A", au))
                else:
                    order.append(("A", au))
                    order.append(("D", du))

            for kind, (t, h) in order:
                ps = pspool.tile([128, UW], F32, tag="u")
                for j in range(2):
                    c0 = h * UW + j * 512
                    nc.tensor.matmul(
                        ps[:, j * 512 : (j + 1) * 512],
                        lhsT_sb[:, t * 128 : (t + 1) * 128],
                        rhs_sb[:, c0 : c0 + 512],
                        start=True,
                        stop=True,
                    )
                if kind == "D":
                    nc.vector.max(cn[:, t * 16 + h * 8 : t * 16 + (h + 1) * 8], ps[:])
                else:
                    sk = skpool.tile([128, UW], F8, tag="sk")
                    nc.scalar.copy(sk[:], ps[:])
                    off = (h - D_UNITS[t]) * UW
                    # per-unit fp8 DMA on the idle Pool DGE
                    nc.gpsimd.dma_start(f8[t, :, off : off + UW], sk[:])

            nc.sync.dma_start(cand[:, :], cn[:])

    nc.compile()
    return nc


def _get_nc():
    if "nc" not in _NC_CACHE:
        _NC_CACHE["nc"] = _build_nc()
    return _NC_CACHE["nc"]


# ---------------- host-side helpers (all float32, mirror reference) ----------


def _sample_descriptors(desc2, kp):
    """Bilinear sample of desc2 (B,C,H,W) at image-space (y,x) kp, L2-normed."""
    b, c, h, w = desc2.shape
    f = np.float32
    y = np.clip(kp[..., 0] / f(GS) - f(0.5), f(0.0), f(h - 1.0)).astype(f)
    x = np.clip(kp[..., 1] / f(GS) - f(0.5), f(0.0), f(w - 1.0)).astype(f)
    y0 = np.clip(np.floor(y), 0, h - 2).astype(np.int64)
    x0 = np.clip(np.floor(x), 0, w - 2).astype(np.int64)
    wy = (y - y0.astype(f))[..., None]
    wx = (x - x0.astype(f))[..., None]
    dmap = desc2.transpose(0, 2, 3, 1).reshape(b, h * w, c)

    def g(yi, xi):
        idx = yi * w + xi
        return np.take_along_axis(dmap, idx[..., None], axis=1)

    v = (
        g(y0, x0) * (1 - wy) * (1 - wx)
        + g(y0, x0 + 1) * (1 - wy) * wx
        + g(y0 + 1, x0) * wy * (1 - wx)
        + g(y0 + 1, x0 + 1) * wy * wx
    )
    n = np.sqrt(np.sum(v * v, axis=-1, keepdims=True)).astype(f)
    return (v / (n + f(1e-8))).astype(f)


def _nearest4(pts):
    """Flat ids (..., 4) of the 4 nearest grid-cell centers, matching the
    reference's top_k over all HW cells (ties -> lower flat id)."""
    f = np.float32
    y = pts[..., 0]
    x = pts[..., 1]
    cy = np.clip(np.floor(y / f(GS)).astype(np.int64), 0, H - 1)
    cx = np.clip(np.floor(x / f(GS)).astype(np.int64), 0, W - 1)
    by = np.clip(cy - 2, 0, H - 5)
    bx = np.clip(cx - 2, 0, W - 5)
    offs = np.arange(5, dtype=np.int64)
    iy = by[..., None] + offs          # (..., 5)
    ix = bx[..., None] + offs
    cyc = (f(GS) * iy + f(GS / 2.0)).astype(f)
    cxc = (f(GS) * ix + f(GS / 2.0)).astype(f)
    dy = y[..., None] - cyc
    dx = x[..., None] - cxc
    d2 = (dy * dy)[..., :, None] + (dx * dx)[..., None, :]   # (..., 5, 5)
    ids = iy[..., :, None] * W + ix[..., None, :]
    d2 = d2.reshape(d2.shape[:-2] + (25,))
    ids = ids.reshape(ids.shape[:-2] + (25,))
    order = np.argsort(d2, axis=-1, kind="stable")[..., :4]
    return np.take_along_axis(ids, order, axis=-1)


def _warp(p, Hm):
    f = np.float32
    xy = p[..., ::-1]
    ph = np.concatenate([xy, np.ones_like(xy[..., :1])], axis=-1)
    wp = np.einsum("bij,bmj->bmi", Hm, ph).astype(f)
    wp = wp[..., :2] / (wp[..., 2:3] + f(1e-8))
    return wp[..., ::-1].astype(f)


def _centers(ids):
    f = np.float32
    yy = (ids // W).astype(f) * f(GS) + f(GS / 2.0)
    xx = (ids % W).astype(f) * f(GS) + f(GS / 2.0)
    return np.stack([yy, xx], axis=-1)


def _topk_smallest_idx(x, k):
    """Indices of k smallest per row, ties -> lower index (lax.top_k order)."""
    return np.argsort(x, axis=-1, kind="stable")[..., :k]


def kernel(kp1, w_kp1, kp1_desc, desc2, homo12):
    global LAST_RESULTS

    f = np.float32
    kp1 = np.asarray(kp1, f)
    w_kp1 = np.asarray(w_kp1, f)
    kp1_desc = np.asarray(kp1_desc, f)
    desc2 = np.asarray(desc2, f)
    homo12 = np.asarray(homo12, f)

    # ---------------- host geometry / small tensors ----------------
    w_kp1_desc = _sample_descriptors(desc2, w_kp1)                  # (B,N,C)
    pos = f(2.0) - f(2.0) * np.einsum("bnc,bnc->bn", kp1_desc, w_kp1_desc)

    cell4 = _nearest4(kp1)                                          # (B,N,4)
    kp1_cells = _centers(cell4.reshape(B, 4 * N))                   # (B,4N,2)
    warped = _warp(kp1_cells, homo12)                               # (B,4N,2)
    wcc = _nearest4(warped)                                         # (B,4N,4)
    ids16 = wcc.reshape(B, N, 16)                                   # neigh cells
    cell4_w = _nearest4(w_kp1)                                      # (B,N,4)

    eqk = cell4[:, :, :, None, None] == cell4[:, None, None, :, :]
    kp1_mask = eqk.sum(axis=(2, 4)).astype(f)                       # (B,N,N)
    eqw = ids16[:, :, :, None, None] == cell4_w[:, None, None, :, :]
    w_kp1_mask = eqw.sum(axis=(2, 4)).astype(f)                     # (B,N,N)

    # ---------------- device run ----------------
    nc = _get_nc()
    bf = ml_dtypes.bfloat16
    desc2_flat = np.ascontiguousarray(desc2.reshape(B, C, HW))
    in_maps = []
    for b in range(B):
        lhsT_b = np.ascontiguousarray(kp1_desc[b].T.astype(bf))
        for s in range(NSHARD):
            in_maps.append(
                {
                    "lhsT": lhsT_b,
                    "rhs": np.ascontiguousarray(
                        desc2_flat[b][:, s * SHW : (s + 1) * SHW].astype(bf)
                    ),
                }
            )
    want_trace = bool(int(os.environ.get("KT_TRACE", "0")))
    try:
        res = run_bass_kernel_spmd(
            nc, in_maps, core_ids=list(range(8)), trace=want_trace
        )
    except ModuleNotFoundError:
        res = run_bass_kernel_spmd(nc, in_maps, core_ids=list(range(8)), trace=False)
    LAST_RESULTS = res
    results = res.results

    # dve_vals[b, n, s, :] : top-8 values per DVE unit (t0-2: 2 units, t3: 1)
    # f8vals[b, t, s]      : fp8 sketch [128, W_t] (as float32)
    dve_vals = np.full((B, N, NSHARD, 16), -np.inf, f)
    f8vals = np.empty((B, RT, NSHARD, 128, F8W), f)
    for ci in range(8):
        b, s = divmod(ci, NSHARD)
        r = results[ci]
        cnd = np.asarray(r["cand"], f)                              # (128,RT*16)
        skv = np.asarray(r["f8"]).astype(f)                         # (RT,128,F8W)
        for t in range(RT):
            rows = slice(t * 128, (t + 1) * 128)
            nu_d = D_UNITS[t]
            dve_vals[b, rows, s, : nu_d * 8] = cnd[:, t * 16 : t * 16 + nu_d * 8]
            f8vals[b, t, s] = skv[t]

    # ---------------- fos merge ----------------
    # exact raw sims of the 16 neighbor (masked) cells
    hwdesc = desc2_flat.transpose(0, 2, 1)                          # (B,HW,C)
    gath = np.take_along_axis(
        hwdesc, ids16.reshape(B, N * 16)[:, :, None], axis=1
    ).reshape(B, N, 16, C)
    vm16 = np.einsum("bnc,bnjc->bnj", kp1_desc, gath).astype(f)     # (B,N,16)

    KF = 96                    # fp8 candidates to exact-evaluate per row
    NOISE = f(0.008)           # bf16-input matmul noise bound on scores
    neg_scores = np.empty((B, N, NUM_NEG), f)
    repair = np.zeros((B, N), bool)

    # Per-row fp8 candidate cells (global ids) + non-candidate upper bound.
    # Row n (in tile t): fp8 region of shard s = cells
    # [s*SHW + D_CELLS[t], (s+1)*SHW) with width Wt = SHW - D_CELLS[t].
    for t in range(RT):
        rows = slice(t * 128, (t + 1) * 128)
        Wt = SHW - D_CELLS[t]
        blk = f8vals[:, t, :, :, :Wt]                               # (B,S,128,Wt)
        blk = blk.transpose(0, 2, 1, 3).reshape(B, 128, NSHARD * Wt)
        # global cell ids for the concatenated fp8 region
        ids_s = (
            np.arange(NSHARD)[:, None] * SHW + D_CELLS[t] + np.arange(Wt)[None, :]
        ).reshape(-1)                                               # (S*Wt,)
        top = np.argpartition(-blk, KF, axis=-1)[..., : KF + 1]
        topv = np.take_along_axis(blk, top, axis=-1)
        ordi = np.argsort(-topv, axis=-1, kind="stable")
        top = np.take_along_axis(top, ordi, axis=-1)
        topv = np.take_along_axis(topv, ordi, axis=-1)
        cand_ids = ids_s[top[..., :KF]]                             # (B,128,KF)
        b_max = topv[..., KF]                                       # max non-cand fp8
        ub = b_max + np.maximum(np.abs(b_max), f(0.25)) * f(0.13) + NOISE

        # exact values of fp8 candidates
        gat = np.take_along_axis(
            hwdesc, cand_ids.reshape(B, 128 * KF)[:, :, None], axis=1
        ).reshape(B, 128, KF, C)
        ev = np.einsum("bnc,bnkc->bnk", kp1_desc[:, rows], gat).astype(f)

        # masked-cell adjustment inside the fp8 region (by index, exact)
        ids16_t = ids16[:, rows]                                    # (B,128,16)
        for bb in range(B):
            for rr in range(128):
                n = t * 128 + rr
                e = ev[bb, rr].copy()
                cid = cand_ids[bb, rr]
                uq, cnts = np.unique(ids16_t[bb, rr], return_counts=True)
                in_dve = (uq % SHW) < D_CELLS[t]
                # fp8-region masked cells present among candidates
                for u, cc in zip(uq[~in_dve], cnts[~in_dve]):
                    hit = np.nonzero(cid == u)[0]
                    if hit.size:
                        e[hit[0]] -= f(2.5) * cc
                # DVE-region masked cells: patch by value or repair
                dv = dve_vals[bb, n].reshape(-1).copy()
                order_mask = np.argsort(-vm16[bb, n])
                for j in order_mask:
                    u = ids16_t[bb, rr, j]
                    if (u % SHW) >= D_CELLS[t]:
                        continue
                    # count of this cell (dedupe: only process first occurrence)
                    if j != np.nonzero(ids16_t[bb, rr] == u)[0][0]:
                        continue
                    cc = int((ids16_t[bb, rr] == u).sum())
                    s_of = u // SHW
                    unit = (u % SHW) // UW
                    u8 = dve_vals[bb, n, s_of, unit * 8 : unit * 8 + 8]
                    vm = vm16[bb, n, j]
                    if vm + NOISE + f(0.004) < u8[7]:
                        continue                    # below the unit's top-8
                    d = np.abs(u8 - vm)
                    hits = np.nonzero(d <= NOISE)[0]
                    if hits.size != 1 or np.sort(d)[1] <= f(2.0) * NOISE:
                        repair[bb, n] = True
                        break
                    dv[s_of * 16 + unit * 8 + hits[0]] -= f(2.5) * cc
                if repair[bb, n]:
                    continue
                pool_v = np.concatenate([e, dv[np.isfinite(dv)]])
                pool_v.sort()
                top16 = pool_v[::-1][:NUM_NEG]
                v16 = top16[-1]
                # certificates
                if ub[bb, rr] >= v16 - f(1e-3):
                    repair[bb, n] = True
                    continue
                unit8 = dve_vals[bb, n, :, 7::8].reshape(-1)        # 8th per unit
                unit8 = unit8[np.isfinite(unit8)]
                if unit8.size and unit8.max() + NOISE >= v16:
                    repair[bb, n] = True
                    continue
                neg_scores[bb, n] = top16

    # exact repair (rare): full masked row on host
    if os.environ.get("KT_DEBUG"):
        print(f"[kernel] repair rows: {int(repair.sum())} / {B * N}")
    rep_idx = np.argwhere(repair)
    if rep_idx.size:
        for bb, n in rep_idx:
            row = hwdesc[bb] @ kp1_desc[bb, n]                      # (HW,)
            np.subtract.at(row, ids16[bb, n], f(2.5))
            neg_scores[bb, n] = np.sort(row)[::-1][:NUM_NEG]

    neg = f(2.0) - f(2.0) * neg_scores                              # (B,N,16)
    fos = np.mean(
        np.maximum(pos[..., None] - neg + f(MARGIN), f(0.0)) ** 2
    ).astype(f)

    # ---------------- sos (host: 6% of total flops) ----------------
    k_sim = (
        f(2.0) - f(2.0) * np.einsum("bnc,bmc->bnm", kp1_desc, kp1_desc)
        + f(5.0) * kp1_mask
    ).astype(f)
    w_sim = (
        f(2.0) - f(2.0) * np.einsum("bnc,bmc->bnm", w_kp1_desc, w_kp1_desc)
        + f(5.0) * w_kp1_mask
    ).astype(f)
    k_ids = _topk_smallest_idx(k_sim, SOS_NEG)                      # (B,N,8)
    w_ids = _topk_smallest_idx(w_sim, SOS_NEG)

    kd = np.take_along_axis(
        kp1_desc, k_ids.reshape(B, N * SOS_NEG)[:, :, None], axis=1
    ).reshape(B, N, SOS_NEG, C)
    wd = np.take_along_axis(
        w_kp1_desc, w_ids.reshape(B, N * SOS_NEG)[:, :, None], axis=1
    ).reshape(B, N, SOS_NEG, C)
    a = f(2.0) - f(2.0) * np.einsum("bnc,bnkc->bnk", kp1_desc, kd)
    bb_ = f(2.0) - f(2.0) * np.einsum("bnc,bnkc->bnk", w_kp1_desc, wd)
    sv = (a - bb_).astype(f)
    sos = np.mean(np.sqrt(np.sum(sv * sv, axis=-1))).astype(f)

    return np.asarray(fos + sos, dtype=np.float32)


# revision 15
# speedup vs baseline: 1.1839x; 1.1378x over previous
"""Trainium2 Bass kernel for nn_HardQuadTripletSOSRLoss.

Sharding: 8 cores = 2 batches x 4 HW-shards (4096 grid cells each).

Device (per core): 32 bf16 matmuls kp1_desc[b].T x desc2-shard -> PSUM f32
scores [512 rows x 4096 cells], drained by two engines in parallel on
disjoint cell ranges:
  - DVE: max8 top-8 values per [128,1024]-cell unit (exact top-8/unit),
  - Act: PSUM -> fp8(e4m3) sketch, DMA'd out for host-side selection.
Host: bilinear sampling, geometry/masks, SOS top-8 (small: 6% of flops),
fp8-candidate exact re-evaluation, certificate-checked merge with exact
row-repair fallback.
"""

import os

import numpy as np
import ml_dtypes

import concourse.mybir as mybir
import concourse.tile as tile
from concourse import bacc
from concourse.bass_utils import run_bass_kernel_spmd

# ---- problem constants (hardcoded per contract) ----
B, N, C, H, W = 2, 512, 128, 128, 128
HW = H * W
GS = 8
NUM_NEG = 16
SOS_NEG = 8
MARGIN = 1.0
NSHARD = 4
SHW = HW // NSHARD          # 4096 cells per shard
RT = N // 128               # 4 row tiles
UW = 1024                   # unit width (cells) = 2 PSUM banks
NU = SHW // UW              # 4 units per shard (h = 0..3)

# Route map: per row-tile t, first D_CELLS[t] cells of the shard go to the
# DVE (exact top-8 per 1024-unit); the rest are fp8-sketched for the host.
D_UNITS = [2, 2, 2, 2]      # units routed to DVE per row tile (h < D_UNITS[t])
D_CELLS = [u * UW for u in D_UNITS]
F8W = SHW - min(D_CELLS)    # fp8 tensor width (max over t): 3072

F32 = mybir.dt.float32
BF16 = mybir.dt.bfloat16
F8 = mybir.dt.float8e4

_NC_CACHE = {}
LAST_RESULTS = None  # BassKernelResults of most recent device run (for test.py)


def _build_nc():
    nc = bacc.Bacc("TRN2", target_bir_lowering=False, debug=False, num_devices=8)

    lhsT = nc.dram_tensor("lhsT", [C, N], BF16, kind="ExternalInput")
    rhs = nc.dram_tensor("rhs", [C, SHW], BF16, kind="ExternalInput")
    cand = nc.dram_tensor("cand", [128, RT * 16], F32, kind="ExternalOutput")
    f8 = nc.dram_tensor("f8", [RT, 128, F8W], F8, kind="ExternalOutput")

    with tile.TileContext(nc) as tc:
        with (
            tc.tile_pool(name="const", bufs=1) as cpool,
            tc.tile_pool(name="cnd", bufs=1) as cndpool,
            tc.tile_pool(name="sk", bufs=2) as skpool,
            tc.tile_pool(name="psum", bufs=4, space="PSUM") as pspool,
        ):
            # Input DMAs spread across sequencers so descriptor gen runs in
            # parallel right after the framework preamble barrier. First
            # slices are small so the first matmul's data lands early.
            lhsT_sb = cpool.tile([C, N], BF16, tag="lhsT")
            nc.sync.dma_start(lhsT_sb[:, 0:128], lhsT[:, 0:128])
            rhs_sb = cpool.tile([C, SHW], BF16, tag="rhs")
            nc.scalar.dma_start(rhs_sb[:, 0:512], rhs[:, 0:512])
            nc.sync.dma_start(lhsT_sb[:, 128:N], lhsT[:, 128:N])
            nc.scalar.dma_start(rhs_sb[:, 512:1024], rhs[:, 512:1024])
            nc.sync.dma_start(rhs_sb[:, 2 * UW : 3 * UW], rhs[:, 2 * UW : 3 * UW])
            nc.scalar.dma_start(rhs_sb[:, UW : 2 * UW], rhs[:, UW : 2 * UW])
            nc.gpsimd.dma_start(rhs_sb[:, 3 * UW : 4 * UW], rhs[:, 3 * UW : 4 * UW])

            # PE clock warm-up: dummy matmuls on a zeroed tile keep the PE
            # continuously busy through the DMA wait so real matmuls run at
            # full p-state. Results land in the first unit's PSUM tile and
            # are reset by its start=True matmul.
            warm = cpool.tile([128, 512], BF16, tag="warm")
            nc.gpsimd.memset(warm[:], 0.0)
            cn = cndpool.tile([128, RT * 16], F32, tag="cn")

            # Interleave DVE-routed and Act-routed units so the engines
            # drain PSUM in parallel; t-major so each row tile's fp8 DMA
            # fires early. Unit = (t, h) covers cells [h*UW, (h+1)*UW)
            # with lhsT rows [t*128, (t+1)*128).
            dve_units = [(t, h) for t in range(RT) for h in (0, 1) if h < D_UNITS[t]]
            act_units = [(t, h) for t in range(RT) for h in (2, 3) if h >= D_UNITS[t]]
            order = []
            for du, au in zip(dve_units, act_units):
                order.append(("D", du))
                order.append(("A", au))

            sk = []
            for t in range(RT):
                sk_t = skpool.tile([128, F8W], F8, tag=f"sk{t}")
                sk.append(sk_t)

            first = True
            act_done = {t: 0 for t in range(RT)}
            for kind, (t, h) in order:
                ps = pspool.tile([128, UW], F32, tag="u")
                if first:
                    for _ in range(12):
                        nc.tensor.matmul(
                            ps[:, 0:512], warm[:, 0:128], warm[:],
                            start=True, stop=True,
                        )
                    first = False
                for j in range(2):
                    c0 = h * UW + j * 512
                    nc.tensor.matmul(
                        ps[:, j * 512 : (j + 1) * 512],
                        lhsT_sb[:, t * 128 : (t + 1) * 128],
                        rhs_sb[:, c0 : c0 + 512],
                        start=True,
                        stop=True,
                    )
                if kind == "D":
                    nc.vector.max(cn[:, t * 16 + h * 8 : t * 16 + (h + 1) * 8], ps[:])
                else:
                    off = (h - D_UNITS[t]) * UW
                    nc.scalar.copy(sk[t][:, off : off + UW], ps[:])
                    act_done[t] += 1
                    if act_done[t] == NU - D_UNITS[t]:
                        wt = (NU - D_UNITS[t]) * UW
                        nc.gpsimd.dma_start(f8[t, :, 0:wt], sk[t][:, 0:wt])

            nc.sync.dma_start(cand[:, :], cn[:])

    nc.compile()
    return nc


def _get_nc():
    if "nc" not in _NC_CACHE:
        _NC_CACHE["nc"] = _build_nc()
    return _NC_CACHE["nc"]


# ---------------- host-side helpers (all float32, mirror reference) ----------


def _sample_descriptors(desc2, kp):
    """Bilinear sample of desc2 (B,C,H,W) at image-space (y,x) kp, L2-normed."""
    b, c, h, w = desc2.shape
    f = np.float32
    y = np.clip(kp[..., 0] / f(GS) - f(0.5), f(0.0), f(h - 1.0)).astype(f)
    x = np.clip(kp[..., 1] / f(GS) - f(0.5), f(0.0), f(w - 1.0)).astype(f)
    y0 = np.clip(np.floor(y), 0, h - 2).astype(np.int64)
    x0 = np.clip(np.floor(x), 0, w - 2).astype(np.int64)
    wy = (y - y0.astype(f))[..., None]
    wx = (x - x0.astype(f))[..., None]
    dmap = desc2.transpose(0, 2, 3, 1).reshape(b, h * w, c)

    def g(yi, xi):
        idx = yi * w + xi
        return np.take_along_axis(dmap, idx[..., None], axis=1)

    v = (
        g(y0, x0) * (1 - wy) * (1 - wx)
        + g(y0, x0 + 1) * (1 - wy) * wx
        + g(y0 + 1, x0) * wy * (1 - wx)
        + g(y0 + 1, x0 + 1) * wy * wx
    )
    n = np.sqrt(np.sum(v * v, axis=-1, keepdims=True)).astype(f)
    return (v / (n + f(1e-8))).astype(f)


def _nearest4(pts):
    """Flat ids (..., 4) of the 4 nearest grid-cell centers, matching the
    reference's top_k over all HW cells (ties -> lower flat id)."""
    f = np.float32
    y = pts[..., 0]
    x = pts[..., 1]
    cy = np.clip(np.floor(y / f(GS)).astype(np.int64), 0, H - 1)
    cx = np.clip(np.floor(x / f(GS)).astype(np.int64), 0, W - 1)
    by = np.clip(cy - 2, 0, H - 5)
    bx = np.clip(cx - 2, 0, W - 5)
    offs = np.arange(5, dtype=np.int64)
    iy = by[..., None] + offs          # (..., 5)
    ix = bx[..., None] + offs
    cyc = (f(GS) * iy + f(GS / 2.0)).astype(f)
    cxc = (f(GS) * ix + f(GS / 2.0)).astype(f)
    dy = y[..., None] - cyc
    dx = x[..., None] - cxc
    d2 = (dy * dy)[..., :, None] + (dx * dx)[..., None, :]   # (..., 5, 5)
    ids = iy[..., :, None] * W + ix[..., None, :]
    d2 = d2.reshape(d2.shape[:-2] + (25,))
    ids = ids.reshape(ids.shape[:-2] + (25,))
    order = np.argsort(d2, axis=-1, kind="stable")[..., :4]
    return np.take_along_axis(ids, order, axis=-1)


def _warp(p, Hm):
    f = np.float32
    xy = p[..., ::-1]
    ph = np.concatenate([xy, np.ones_like(xy[..., :1])], axis=-1)
    wp = np.einsum("bij,bmj->bmi", Hm, ph).astype(f)
    wp = wp[..., :2] / (wp[..., 2:3] + f(1e-8))
    return wp[..., ::-1].astype(f)


def _centers(ids):
    f = np.float32
    yy = (ids // W).astype(f) * f(GS) + f(GS / 2.0)
    xx = (ids % W).astype(f) * f(GS) + f(GS / 2.0)
    return np.stack([yy, xx], axis=-1)


def _topk_smallest_idx(x, k):
    """Indices of k smallest per row, ties -> lower index (lax.top_k order)."""
    return np.argsort(x, axis=-1, kind="stable")[..., :k]


def kernel(kp1, w_kp1, kp1_desc, desc2, homo12):
    global LAST_RESULTS

    f = np.float32
    kp1 = np.asarray(kp1, f)
    w_kp1 = np.asarray(w_kp1, f)
    kp1_desc = np.asarray(kp1_desc, f)
    desc2 = np.asarray(desc2, f)
    homo12 = np.asarray(homo12, f)

    # ---------------- host geometry / small tensors ----------------
    w_kp1_desc = _sample_descriptors(desc2, w_kp1)                  # (B,N,C)
    pos = f(2.0) - f(2.0) * np.einsum("bnc,bnc->bn", kp1_desc, w_kp1_desc)

    cell4 = _nearest4(kp1)                                          # (B,N,4)
    kp1_cells = _centers(cell4.reshape(B, 4 * N))                   # (B,4N,2)
    warped = _warp(kp1_cells, homo12)                               # (B,4N,2)
    wcc = _nearest4(warped)                                         # (B,4N,4)
    ids16 = wcc.reshape(B, N, 16)                                   # neigh cells
    cell4_w = _nearest4(w_kp1)                                      # (B,N,4)

    eqk = cell4[:, :, :, None, None] == cell4[:, None, None, :, :]
    kp1_mask = eqk.sum(axis=(2, 4)).astype(f)                       # (B,N,N)
    eqw = ids16[:, :, :, None, None] == cell4_w[:, None, None, :, :]
    w_kp1_mask = eqw.sum(axis=(2, 4)).astype(f)                     # (B,N,N)

    # ---------------- device run ----------------
    nc = _get_nc()
    bf = ml_dtypes.bfloat16
    desc2_flat = np.ascontiguousarray(desc2.reshape(B, C, HW))
    in_maps = []
    for b in range(B):
        lhsT_b = np.ascontiguousarray(kp1_desc[b].T.astype(bf))
        for s in range(NSHARD):
            in_maps.append(
                {
                    "lhsT": lhsT_b,
                    "rhs": np.ascontiguousarray(
                        desc2_flat[b][:, s * SHW : (s + 1) * SHW].astype(bf)
                    ),
                }
            )
    want_trace = bool(int(os.environ.get("KT_TRACE", "0")))
    try:
        res = run_bass_kernel_spmd(
            nc, in_maps, core_ids=list(range(8)), trace=want_trace
        )
    except ModuleNotFoundError:
        res = run_bass_kernel_spmd(nc, in_maps, core_ids=list(range(8)), trace=False)
    LAST_RESULTS = res
    results = res.results

    # dve_vals[b, n, s, :] : top-8 values per DVE unit (t0-2: 2 units, t3: 1)
    # f8vals[b, t, s]      : fp8 sketch [128, W_t] (as float32)
    dve_vals = np.full((B, N, NSHARD, 16), -np.inf, f)
    f8vals = np.empty((B, RT, NSHARD, 128, F8W), f)
    for ci in range(8):
        b, s = divmod(ci, NSHARD)
        r = results[ci]
        cnd = np.asarray(r["cand"], f)                              # (128,RT*16)
        skv = np.asarray(r["f8"]).astype(f)                         # (RT,128,F8W)
        for t in range(RT):
            rows = slice(t * 128, (t + 1) * 128)
            nu_d = D_UNITS[t]
            dve_vals[b, rows, s, : nu_d * 8] = cnd[:, t * 16 : t * 16 + nu_d * 8]
            f8vals[b, t, s] = skv[t]

    # ---------------- fos merge ----------------
    # exact raw sims of the 16 neighbor (masked) cells
    hwdesc = desc2_flat.transpose(0, 2, 1)                          # (B,HW,C)
    gath = np.take_along_axis(
        hwdesc, ids16.reshape(B, N * 16)[:, :, None], axis=1
    ).reshape(B, N, 16, C)
    vm16 = np.einsum("bnc,bnjc->bnj", kp1_desc, gath).astype(f)     # (B,N,16)

    KF = 96                    # fp8 candidates to exact-evaluate per row
    NOISE = f(0.008)           # bf16-input matmul noise bound on scores
    neg_scores = np.empty((B, N, NUM_NEG), f)
    repair = np.zeros((B, N), bool)

    # Per-row fp8 candidate cells (global ids) + non-candidate upper bound.
    # Row n (in tile t): fp8 region of shard s = cells
    # [s*SHW + D_CELLS[t], (s+1)*SHW) with width Wt = SHW - D_CELLS[t].
    for t in range(RT):
        rows = slice(t * 128, (t + 1) * 128)
        Wt = SHW - D_CELLS[t]
        blk = f8vals[:, t, :, :, :Wt]                               # (B,S,128,Wt)
        blk = blk.transpose(0, 2, 1, 3).reshape(B, 128, NSHARD * Wt)
        # global cell ids for the concatenated fp8 region
        ids_s = (
            np.arange(NSHARD)[:, None] * SHW + D_CELLS[t] + np.arange(Wt)[None, :]
        ).reshape(-1)                                               # (S*Wt,)
        top = np.argpartition(-blk, KF, axis=-1)[..., : KF + 1]
        topv = np.take_along_axis(blk, top, axis=-1)
        ordi = np.argsort(-topv, axis=-1, kind="stable")
        top = np.take_along_axis(top, ordi, axis=-1)
        topv = np.take_along_axis(topv, ordi, axis=-1)
        cand_ids = ids_s[top[..., :KF]]                             # (B,128,KF)
        b_max = topv[..., KF]                                       # max non-cand fp8
        ub = b_max + np.maximum(np.abs(b_max), f(0.25)) * f(0.13) + NOISE

        # exact values of fp8 candidates
        gat = np.take_along_axis(
            hwdesc, cand_ids.reshape(B, 128 * KF)[:, :, None], axis=1
        ).reshape(B, 128, KF, C)
        ev = np.einsum("bnc,bnkc->bnk", kp1_desc[:, rows], gat).astype(f)

        # masked-cell adjustment inside the fp8 region (by index, exact)
        ids16_t = ids16[:, rows]                                    # (B,128,16)
        for bb in range(B):
            for rr in range(128):
                n = t * 128 + rr
                e = ev[bb, rr].copy()
                cid = cand_ids[bb, rr]
                uq, cnts = np.unique(ids16_t[bb, rr], return_counts=True)
                in_dve = (uq % SHW) < D_CELLS[t]
                # fp8-region masked cells present among candidates
                for u, cc in zip(uq[~in_dve], cnts[~in_dve]):
                    hit = np.nonzero(cid == u)[0]
                    if hit.size:
                        e[hit[0]] -= f(2.5) * cc
                # DVE-region masked cells: patch by value or repair
                dv = dve_vals[bb, n].reshape(-1).copy()
                order_mask = np.argsort(-vm16[bb, n])
                for j in order_mask:
                    u = ids16_t[bb, rr, j]
                    if (u % SHW) >= D_CELLS[t]:
                        continue
                    # count of this cell (dedupe: only process first occurrence)
                    if j != np.nonzero(ids16_t[bb, rr] == u)[0][0]:
                        continue
                    cc = int((ids16_t[bb, rr] == u).sum())
                    s_of = u // SHW
                    unit = (u % SHW) // UW
                    u8 = dve_vals[bb, n, s_of, unit * 8 : unit * 8 + 8]
                    vm = vm16[bb, n, j]
                    if vm + NOISE + f(0.004) < u8[7]:
                        continue                    # below the unit's top-8
                    d = np.abs(u8 - vm)
                    hits = np.nonzero(d <= NOISE)[0]
                    if hits.size != 1 or np.sort(d)[1] <= f(2.0) * NOISE:
                        repair[bb, n] = True
                        break
                    dv[s_of * 16 + unit * 8 + hits[0]] -= f(2.5) * cc
                if repair[bb, n]:
                    continue
                pool_v = np.concatenate([e, dv[np.isfinite(dv)]])
                pool_v.sort()
                top16 = pool_v[::-1][:NUM_NEG]
                v16 = top16[-1]
                # certificates
                if ub[bb, rr] >= v16 - f(1e-3):
                    repair[bb, n] = True
                    continue
                unit8 = dve_vals[bb, n, :, 7::8].reshape(-1)        # 8th per unit
                unit8 = unit8[np.isfinite(unit8)]
                if unit8.size and unit8.max() + NOISE >= v16:
                    repair[bb, n] = True
                    continue
                neg_scores[bb, n] = top16

    # exact repair (rare): full masked row on host
    if os.environ.get("KT_DEBUG"):
        print(f"[kernel] repair rows: {int(repair.sum())} / {B * N}")
    rep_idx = np.argwhere(repair)
    if rep_idx.size:
        for bb, n in rep_idx:
            row = hwdesc[bb] @ kp1_desc[bb, n]                      # (HW,)
            np.subtract.at(row, ids16[bb, n], f(2.5))
            neg_scores[bb, n] = np.sort(row)[::-1][:NUM_NEG]

    neg = f(2.0) - f(2.0) * neg_scores                              # (B,N,16)
    fos = np.mean(
        np.maximum(pos[..., None] - neg + f(MARGIN), f(0.0)) ** 2
    ).astype(f)

    # ---------------- sos (host: 6% of total flops) ----------------
    k_sim = (
        f(2.0) - f(2.0) * np.einsum("bnc,bmc->bnm", kp1_desc, kp1_desc)
        + f(5.0) * kp1_mask
    ).astype(f)
    w_sim = (
        f(2.0) - f(2.0) * np.einsum("bnc,bmc->bnm", w_kp1_desc, w_kp1_desc)
        + f(5.0) * w_kp1_mask
    ).astype(f)
    k_ids = _topk_smallest_idx(k_sim, SOS_NEG)                      # (B,N,8)
    w_ids = _topk_smallest_idx(w_sim, SOS_NEG)

    kd = np.take_along_axis(
        kp1_desc, k_ids.reshape(B, N * SOS_NEG)[:, :, None], axis=1
    ).reshape(B, N, SOS_NEG, C)
    wd = np.take_along_axis(
        w_kp1_desc, w_ids.reshape(B, N * SOS_NEG)[:, :, None], axis=1
    ).reshape(B, N, SOS_NEG, C)
    a = f(2.0) - f(2.0) * np.einsum("bnc,bnkc->bnk", kp1_desc, kd)
    bb_ = f(2.0) - f(2.0) * np.einsum("bnc,bnkc->bnk", w_kp1_desc, wd)
    sv = (a - bb_).astype(f)
    sos = np.mean(np.sqrt(np.sum(sv * sv, axis=-1))).astype(f)

    return np.asarray(fos + sos, dtype=np.float32)


# revision 17
# speedup vs baseline: 1.2612x; 1.0653x over previous
"""Trainium2 Bass kernel for nn_HardQuadTripletSOSRLoss.

Sharding: 8 cores = 2 batches x 4 HW-shards (4096 grid cells each).

Device (per core): 32 bf16 matmuls kp1_desc[b].T x desc2-shard -> PSUM f32
scores [512 rows x 4096 cells], drained by two engines in parallel on
disjoint cell ranges:
  - DVE: max8 top-8 values per [128,1024]-cell unit (exact top-8/unit),
  - Act: PSUM -> fp8(e4m3) sketch, DMA'd out for host-side selection.
Host: bilinear sampling, geometry/masks, SOS top-8 (small: 6% of flops),
fp8-candidate exact re-evaluation, certificate-checked merge with exact
row-repair fallback.
"""

import os

import numpy as np
import ml_dtypes

import concourse.mybir as mybir
import concourse.tile as tile
from concourse import bacc
from concourse.bass_utils import run_bass_kernel_spmd

# ---- problem constants (hardcoded per contract) ----
B, N, C, H, W = 2, 512, 128, 128, 128
HW = H * W
GS = 8
NUM_NEG = 16
SOS_NEG = 8
MARGIN = 1.0
NSHARD = 4
SHW = HW // NSHARD          # 4096 cells per shard
RT = N // 128               # 4 row tiles
UW = 1024                   # unit width (cells) = 2 PSUM banks
NU = SHW // UW              # 4 units per shard (h = 0..3)

# Route map: per row-tile t, first D_CELLS[t] cells of the shard go to the
# DVE (exact top-8 per 1024-unit); the rest are fp8-sketched for the host.
D_UNITS = [2, 2, 2, 2]      # units routed to DVE per row tile (h < D_UNITS[t])
D_CELLS = [u * UW for u in D_UNITS]
F8W = SHW - min(D_CELLS)    # fp8 tensor width (max over t): 3072

F32 = mybir.dt.float32
BF16 = mybir.dt.bfloat16
F8 = mybir.dt.float8e4

_NC_CACHE = {}
LAST_RESULTS = None  # BassKernelResults of most recent device run (for test.py)


def _build_nc():
    nc = bacc.Bacc("TRN2", target_bir_lowering=False, debug=False, num_devices=8)

    lhsT = nc.dram_tensor("lhsT", [C, N], BF16, kind="ExternalInput")
    rhs = nc.dram_tensor("rhs", [C, SHW], BF16, kind="ExternalInput")
    cand = nc.dram_tensor("cand", [128, RT * 16], F32, kind="ExternalOutput")
    f8 = nc.dram_tensor("f8", [RT, 128, F8W], F8, kind="ExternalOutput")

    with tile.TileContext(nc) as tc:
        with (
            tc.tile_pool(name="const", bufs=1) as cpool,
            tc.tile_pool(name="cnd", bufs=1) as cndpool,
            tc.tile_pool(name="sk", bufs=2) as skpool,
            tc.tile_pool(name="psum", bufs=4, space="PSUM") as pspool,
        ):
            # Input DMAs spread across sequencers so descriptor gen runs in
            # parallel right after the framework preamble barrier. First
            # slices are small so the first matmul's data lands early.
            lhsT_sb = cpool.tile([C, N], BF16, tag="lhsT")
            nc.sync.dma_start(lhsT_sb[:, 0:128], lhsT[:, 0:128])
            rhs_sb = cpool.tile([C, SHW], BF16, tag="rhs")
            nc.scalar.dma_start(rhs_sb[:, 0:512], rhs[:, 0:512])
            nc.sync.dma_start(lhsT_sb[:, 128:N], lhsT[:, 128:N])
            nc.scalar.dma_start(rhs_sb[:, 512:1024], rhs[:, 512:1024])
            nc.sync.dma_start(rhs_sb[:, 2 * UW : 3 * UW], rhs[:, 2 * UW : 3 * UW])
            nc.scalar.dma_start(rhs_sb[:, UW : 2 * UW], rhs[:, UW : 2 * UW])
            nc.gpsimd.dma_start(rhs_sb[:, 3 * UW : 4 * UW], rhs[:, 3 * UW : 4 * UW])

            # PE clock warm-up: dummy matmuls on a zeroed tile keep the PE
            # continuously busy through the DMA wait so real matmuls run at
            # full p-state. Results land in the first unit's PSUM tile and
            # are reset by its start=True matmul.
            warm = cpool.tile([128, 512], BF16, tag="warm")
            nc.vector.memset(warm[:], 0.0)
            cn = cndpool.tile([128, RT * 16], F32, tag="cn")

            # Interleave DVE-routed and Act-routed units so the engines
            # drain PSUM in parallel; t-major so each row tile's fp8 DMA
            # fires early. Unit = (t, h) covers cells [h*UW, (h+1)*UW)
            # with lhsT rows [t*128, (t+1)*128).
            dve_units = [(t, h) for t in range(RT) for h in (0, 1) if h < D_UNITS[t]]
            act_units = [(t, h) for t in range(RT) for h in (2, 3) if h >= D_UNITS[t]]
            order = []
            for du, au in zip(dve_units, act_units):
                order.append(("D", du))
                order.append(("A", au))

            sk = []
            for t in range(RT):
                sk_t = skpool.tile([128, F8W], F8, tag=f"sk{t}")
                sk.append(sk_t)

            first = True
            act_done = {t: 0 for t in range(RT)}
            for kind, (t, h) in order:
                ps = pspool.tile([128, UW], F32, tag="u")
                if first:
                    for _ in range(10):
                        nc.tensor.matmul(
                            ps[:, 0:512], warm[:, 0:128], warm[:],
                            start=True, stop=True,
                        )
                    first = False
                for j in range(2):
                    c0 = h * UW + j * 512
                    nc.tensor.matmul(
                        ps[:, j * 512 : (j + 1) * 512],
                        lhsT_sb[:, t * 128 : (t + 1) * 128],
                        rhs_sb[:, c0 : c0 + 512],
                        start=True,
                        stop=True,
                    )
                if kind == "D":
                    nc.vector.max(cn[:, t * 16 + h * 8 : t * 16 + (h + 1) * 8], ps[:])
                else:
                    off = (h - D_UNITS[t]) * UW
                    nc.scalar.copy(sk[t][:, off : off + UW], ps[:])
                    act_done[t] += 1
                    if act_done[t] == NU - D_UNITS[t]:
                        wt = (NU - D_UNITS[t]) * UW
                        nc.gpsimd.dma_start(f8[t, :, 0:wt], sk[t][:, 0:wt])

            nc.sync.dma_start(cand[:, :], cn[:])

    nc.compile()
    return nc


def _get_nc():
    if "nc" not in _NC_CACHE:
        _NC_CACHE["nc"] = _build_nc()
    return _NC_CACHE["nc"]


# ---------------- host-side helpers (all float32, mirror reference) ----------


def _sample_descriptors(desc2, kp):
    """Bilinear sample of desc2 (B,C,H,W) at image-space (y,x) kp, L2-normed."""
    b, c, h, w = desc2.shape
    f = np.float32
    y = np.clip(kp[..., 0] / f(GS) - f(0.5), f(0.0), f(h - 1.0)).astype(f)
    x = np.clip(kp[..., 1] / f(GS) - f(0.5), f(0.0), f(w - 1.0)).astype(f)
    y0 = np.clip(np.floor(y), 0, h - 2).astype(np.int64)
    x0 = np.clip(np.floor(x), 0, w - 2).astype(np.int64)
    wy = (y - y0.astype(f))[..., None]
    wx = (x - x0.astype(f))[..., None]
    dmap = desc2.transpose(0, 2, 3, 1).reshape(b, h * w, c)

    def g(yi, xi):
        idx = yi * w + xi
        return np.take_along_axis(dmap, idx[..., None], axis=1)

    v = (
        g(y0, x0) * (1 - wy) * (1 - wx)
        + g(y0, x0 + 1) * (1 - wy) * wx
        + g(y0 + 1, x0) * wy * (1 - wx)
        + g(y0 + 1, x0 + 1) * wy * wx
    )
    n = np.sqrt(np.sum(v * v, axis=-1, keepdims=True)).astype(f)
    return (v / (n + f(1e-8))).astype(f)


def _nearest4(pts):
    """Flat ids (..., 4) of the 4 nearest grid-cell centers, matching the
    reference's top_k over all HW cells (ties -> lower flat id)."""
    f = np.float32
    y = pts[..., 0]
    x = pts[..., 1]
    cy = np.clip(np.floor(y / f(GS)).astype(np.int64), 0, H - 1)
    cx = np.clip(np.floor(x / f(GS)).astype(np.int64), 0, W - 1)
    by = np.clip(cy - 2, 0, H - 5)
    bx = np.clip(cx - 2, 0, W - 5)
    offs = np.arange(5, dtype=np.int64)
    iy = by[..., None] + offs          # (..., 5)
    ix = bx[..., None] + offs
    cyc = (f(GS) * iy + f(GS / 2.0)).astype(f)
    cxc = (f(GS) * ix + f(GS / 2.0)).astype(f)
    dy = y[..., None] - cyc
    dx = x[..., None] - cxc
    d2 = (dy * dy)[..., :, None] + (dx * dx)[..., None, :]   # (..., 5, 5)
    ids = iy[..., :, None] * W + ix[..., None, :]
    d2 = d2.reshape(d2.shape[:-2] + (25,))
    ids = ids.reshape(ids.shape[:-2] + (25,))
    order = np.argsort(d2, axis=-1, kind="stable")[..., :4]
    return np.take_along_axis(ids, order, axis=-1)


def _warp(p, Hm):
    f = np.float32
    xy = p[..., ::-1]
    ph = np.concatenate([xy, np.ones_like(xy[..., :1])], axis=-1)
    wp = np.einsum("bij,bmj->bmi", Hm, ph).astype(f)
    wp = wp[..., :2] / (wp[..., 2:3] + f(1e-8))
    return wp[..., ::-1].astype(f)


def _centers(ids):
    f = np.float32
    yy = (ids // W).astype(f) * f(GS) + f(GS / 2.0)
    xx = (ids % W).astype(f) * f(GS) + f(GS / 2.0)
    return np.stack([yy, xx], axis=-1)


def _topk_smallest_idx(x, k):
    """Indices of k smallest per row, ties -> lower index (lax.top_k order)."""
    return np.argsort(x, axis=-1, kind="stable")[..., :k]


def kernel(kp1, w_kp1, kp1_desc, desc2, homo12):
    global LAST_RESULTS

    f = np.float32
    kp1 = np.asarray(kp1, f)
    w_kp1 = np.asarray(w_kp1, f)
    kp1_desc = np.asarray(kp1_desc, f)
    desc2 = np.asarray(desc2, f)
    homo12 = np.asarray(homo12, f)

    # ---------------- host geometry / small tensors ----------------
    w_kp1_desc = _sample_descriptors(desc2, w_kp1)                  # (B,N,C)
    pos = f(2.0) - f(2.0) * np.einsum("bnc,bnc->bn", kp1_desc, w_kp1_desc)

    cell4 = _nearest4(kp1)                                          # (B,N,4)
    kp1_cells = _centers(cell4.reshape(B, 4 * N))                   # (B,4N,2)
    warped = _warp(kp1_cells, homo12)                               # (B,4N,2)
    wcc = _nearest4(warped)                                         # (B,4N,4)
    ids16 = wcc.reshape(B, N, 16)                                   # neigh cells
    cell4_w = _nearest4(w_kp1)                                      # (B,N,4)

    eqk = cell4[:, :, :, None, None] == cell4[:, None, None, :, :]
    kp1_mask = eqk.sum(axis=(2, 4)).astype(f)                       # (B,N,N)
    eqw = ids16[:, :, :, None, None] == cell4_w[:, None, None, :, :]
    w_kp1_mask = eqw.sum(axis=(2, 4)).astype(f)                     # (B,N,N)

    # ---------------- device run ----------------
    nc = _get_nc()
    bf = ml_dtypes.bfloat16
    desc2_flat = np.ascontiguousarray(desc2.reshape(B, C, HW))
    in_maps = []
    for b in range(B):
        lhsT_b = np.ascontiguousarray(kp1_desc[b].T.astype(bf))
        for s in range(NSHARD):
            in_maps.append(
                {
                    "lhsT": lhsT_b,
                    "rhs": np.ascontiguousarray(
                        desc2_flat[b][:, s * SHW : (s + 1) * SHW].astype(bf)
                    ),
                }
            )
    want_trace = bool(int(os.environ.get("KT_TRACE", "0")))
    try:
        res = run_bass_kernel_spmd(
            nc, in_maps, core_ids=list(range(8)), trace=want_trace
        )
    except ModuleNotFoundError:
        res = run_bass_kernel_spmd(nc, in_maps, core_ids=list(range(8)), trace=False)
    LAST_RESULTS = res
    results = res.results

    # dve_vals[b, n, s, :] : top-8 values per DVE unit (t0-2: 2 units, t3: 1)
    # f8vals[b, t, s]      : fp8 sketch [128, W_t] (as float32)
    dve_vals = np.full((B, N, NSHARD, 16), -np.inf, f)
    f8vals = np.empty((B, RT, NSHARD, 128, F8W), f)
    for ci in range(8):
        b, s = divmod(ci, NSHARD)
        r = results[ci]
        cnd = np.asarray(r["cand"], f)                              # (128,RT*16)
        skv = np.asarray(r["f8"]).astype(f)                         # (RT,128,F8W)
        for t in range(RT):
            rows = slice(t * 128, (t + 1) * 128)
            nu_d = D_UNITS[t]
            dve_vals[b, rows, s, : nu_d * 8] = cnd[:, t * 16 : t * 16 + nu_d * 8]
            f8vals[b, t, s] = skv[t]

    # ---------------- fos merge ----------------
    # exact raw sims of the 16 neighbor (masked) cells
    hwdesc = desc2_flat.transpose(0, 2, 1)                          # (B,HW,C)
    gath = np.take_along_axis(
        hwdesc, ids16.reshape(B, N * 16)[:, :, None], axis=1
    ).reshape(B, N, 16, C)
    vm16 = np.einsum("bnc,bnjc->bnj", kp1_desc, gath).astype(f)     # (B,N,16)

    KF = 96                    # fp8 candidates to exact-evaluate per row
    NOISE = f(0.008)           # bf16-input matmul noise bound on scores
    neg_scores = np.empty((B, N, NUM_NEG), f)
    repair = np.zeros((B, N), bool)

    # Per-row fp8 candidate cells (global ids) + non-candidate upper bound.
    # Row n (in tile t): fp8 region of shard s = cells
    # [s*SHW + D_CELLS[t], (s+1)*SHW) with width Wt = SHW - D_CELLS[t].
    for t in range(RT):
        rows = slice(t * 128, (t + 1) * 128)
        Wt = SHW - D_CELLS[t]
        blk = f8vals[:, t, :, :, :Wt]                               # (B,S,128,Wt)
        blk = blk.transpose(0, 2, 1, 3).reshape(B, 128, NSHARD * Wt)
        # global cell ids for the concatenated fp8 region
        ids_s = (
            np.arange(NSHARD)[:, None] * SHW + D_CELLS[t] + np.arange(Wt)[None, :]
        ).reshape(-1)                                               # (S*Wt,)
        top = np.argpartition(-blk, KF, axis=-1)[..., : KF + 1]
        topv = np.take_along_axis(blk, top, axis=-1)
        ordi = np.argsort(-topv, axis=-1, kind="stable")
        top = np.take_along_axis(top, ordi, axis=-1)
        topv = np.take_along_axis(topv, ordi, axis=-1)
        cand_ids = ids_s[top[..., :KF]]                             # (B,128,KF)
        b_max = topv[..., KF]                                       # max non-cand fp8
        ub = b_max + np.maximum(np.abs(b_max), f(0.25)) * f(0.13) + NOISE

        # exact values of fp8 candidates
        gat = np.take_along_axis(
            hwdesc, cand_ids.reshape(B, 128 * KF)[:, :, None], axis=1
        ).reshape(B, 128, KF, C)
        ev = np.einsum("bnc,bnkc->bnk", kp1_desc[:, rows], gat).astype(f)

        # masked-cell adjustment inside the fp8 region (by index, exact)
        ids16_t = ids16[:, rows]                                    # (B,128,16)
        for bb in range(B):
            for rr in range(128):
                n = t * 128 + rr
                e = ev[bb, rr].copy()
                cid = cand_ids[bb, rr]
                uq, cnts = np.unique(ids16_t[bb, rr], return_counts=True)
                in_dve = (uq % SHW) < D_CELLS[t]
                # fp8-region masked cells present among candidates
                for u, cc in zip(uq[~in_dve], cnts[~in_dve]):
                    hit = np.nonzero(cid == u)[0]
                    if hit.size:
                        e[hit[0]] -= f(2.5) * cc
                # DVE-region masked cells: patch by value or repair
                dv = dve_vals[bb, n].reshape(-1).copy()
                order_mask = np.argsort(-vm16[bb, n])
                for j in order_mask:
                    u = ids16_t[bb, rr, j]
                    if (u % SHW) >= D_CELLS[t]:
                        continue
                    # count of this cell (dedupe: only process first occurrence)
                    if j != np.nonzero(ids16_t[bb, rr] == u)[0][0]:
                        continue
                    cc = int((ids16_t[bb, rr] == u).sum())
                    s_of = u // SHW
                    unit = (u % SHW) // UW
                    u8 = dve_vals[bb, n, s_of, unit * 8 : unit * 8 + 8]
                    vm = vm16[bb, n, j]
                    if vm + NOISE + f(0.004) < u8[7]:
                        continue                    # below the unit's top-8
                    d = np.abs(u8 - vm)
                    hits = np.nonzero(d <= NOISE)[0]
                    if hits.size != 1 or np.sort(d)[1] <= f(2.0) * NOISE:
                        repair[bb, n] = True
                        break
                    dv[s_of * 16 + unit * 8 + hits[0]] -= f(2.5) * cc
                if repair[bb, n]:
                    continue
                pool_v = np.concatenate([e, dv[np.isfinite(dv)]])
                pool_v.sort()
                top16 = pool_v[::-1][:NUM_NEG]
                v16 = top16[-1]
                # certificates
                if ub[bb, rr] >= v16 - f(1e-3):
                    repair[bb, n] = True
                    continue
                unit8 = dve_vals[bb, n, :, 7::8].reshape(-1)        # 8th per unit
                unit8 = unit8[np.isfinite(unit8)]
                if unit8.size and unit8.max() + NOISE >= v16:
                    repair[bb, n] = True
                    continue
                neg_scores[bb, n] = top16

    # exact repair (rare): full masked row on host
    if os.environ.get("KT_DEBUG"):
        print(f"[kernel] repair rows: {int(repair.sum())} / {B * N}")
    rep_idx = np.argwhere(repair)
    if rep_idx.size:
        for bb, n in rep_idx:
            row = hwdesc[bb] @ kp1_desc[bb, n]                      # (HW,)
            np.subtract.at(row, ids16[bb, n], f(2.5))
            neg_scores[bb, n] = np.sort(row)[::-1][:NUM_NEG]

    neg = f(2.0) - f(2.0) * neg_scores                              # (B,N,16)
    fos = np.mean(
        np.maximum(pos[..., None] - neg + f(MARGIN), f(0.0)) ** 2
    ).astype(f)

    # ---------------- sos (host: 6% of total flops) ----------------
    k_sim = (
        f(2.0) - f(2.0) * np.einsum("bnc,bmc->bnm", kp1_desc, kp1_desc)
        + f(5.0) * kp1_mask
    ).astype(f)
    w_sim = (
        f(2.0) - f(2.0) * np.einsum("bnc,bmc->bnm", w_kp1_desc, w_kp1_desc)
        + f(5.0) * w_kp1_mask
    ).astype(f)
    k_ids = _topk_smallest_idx(k_sim, SOS_NEG)                      # (B,N,8)
    w_ids = _topk_smallest_idx(w_sim, SOS_NEG)

    kd = np.take_along_axis(
        kp1_desc, k_ids.reshape(B, N * SOS_NEG)[:, :, None], axis=1
    ).reshape(B, N, SOS_NEG, C)
    wd = np.take_along_axis(
        w_kp1_desc, w_ids.reshape(B, N * SOS_NEG)[:, :, None], axis=1
    ).reshape(B, N, SOS_NEG, C)
    a = f(2.0) - f(2.0) * np.einsum("bnc,bnkc->bnk", kp1_desc, kd)
    bb_ = f(2.0) - f(2.0) * np.einsum("bnc,bnkc->bnk", w_kp1_desc, wd)
    sv = (a - bb_).astype(f)
    sos = np.mean(np.sqrt(np.sum(sv * sv, axis=-1))).astype(f)

    return np.asarray(fos + sos, dtype=np.float32)
